# revision 2
# baseline (speedup 1.0000x reference)
"""Masked multi-head attention for Trainium2, SPMD across 8 NeuronCores.

Problem: Q,K,V [4,16,2048,64] f32, mask [1,1,2048,2048] bool (True = masked).
  out = softmax(QK^T/sqrt(64) masked) @ V

Sharding: B*H = 64 (batch, head) pairs -> 8 per core (data + head parallel,
no cross-core communication). Each core runs the identical program on its
8 heads.

Per-core algorithm (S=2048, D=64), all matmul compute in fp16 with f32 PSUM
accumulation:
  - Host passes Q,K transposed per head ([64, 2048], d-major) so the
    contraction dim (d) lands on SBUF partitions; V natural; the mask
    transposed as uint8 keep-flags (1 = keep).
  - For each head, for each q-chunk of 1024, for each k-tile of 128:
      S^T[k,q] = (K^T block [64,128]).T @ Q^T [64,1024]      (TensorE)
      P^T = exp(0.125 * S^T)  f32 PSUM -> fp16 SBUF          (ScalarE LUT)
      P^T *= maskT[k,q]  (0/1 fp16)                          (VectorE 2x)
      O^T[65,1024] += (V'[128,65]).T @ P^T  (V' = [V | 1])   (TensorE)
    The ones column of V' makes O^T row 64 the softmax denominator.
  - Epilogue per q-chunk: copy O^T to SBUF, PE-transpose 128-col blocks,
    out[q, d] = O^T.T[q, 0:64] * (1 / O^T.T[q, 64]), DMA to DRAM.
"""

import numpy as np

import concourse.bass as bass
import concourse.mybir as mybir
import concourse.tile as tile
from concourse import bacc
from concourse.bass import ts
from concourse.bass_utils import run_bass_kernel_spmd
from concourse.masks import make_identity

F32 = mybir.dt.float32
F16 = mybir.dt.float16
U8 = mybir.dt.uint8

N_CORES = 8
B, H, S, D = 4, 16, 2048, 64
HPC = B * H // N_CORES  # heads per core = 8
KT = S // 128  # 16 k-tiles
QC = 2  # q-chunks of 1024
QCS = S // QC  # 1024
JT = QCS // 128  # 8 epilogue blocks per q-chunk
SCALE = 1.0 / np.sqrt(D)


def build_nc():
    nc = bacc.Bacc("TRN2", target_bir_lowering=False)

    qt_d = nc.declare_dram_parameter("qt", [HPC, D, S], F32, isOutput=False)
    kt_d = nc.declare_dram_parameter("kt", [HPC, D, S], F32, isOutput=False)
    v_d = nc.declare_dram_parameter("v", [HPC, S, D], F32, isOutput=False)
    m_d = nc.declare_dram_parameter("maskt", [S, S], U8, isOutput=False)
    o_d = nc.declare_dram_parameter("o", [HPC, S, D], F32, isOutput=True)

    with tile.TileContext(nc) as tc:
        with (
            tc.tile_pool(name="const", bufs=1) as constp,
            tc.tile_pool(name="mask", bufs=1) as maskp,
            tc.tile_pool(name="stage", bufs=2) as stagep,
            tc.tile_pool(name="head", bufs=2) as headp,
            tc.tile_pool(name="pt", bufs=4) as ptp,
            tc.tile_pool(name="ep", bufs=3) as epp,
            tc.tile_pool(name="st", bufs=2, space="PSUM") as stp,
            tc.tile_pool(name="ot", bufs=1, space="PSUM") as otp,
            tc.tile_pool(name="tp", bufs=2, space="PSUM") as tpp,
        ):
            ident = constp.tile([128, 128], F32)
            make_identity(nc, ident[:, :])

            # mask: DMA u8 (transposed on host), cast to fp16 0/1 on GpSimd
            m16 = maskp.tile([128, KT, S], F16)
            m_ap = m_d.ap().rearrange("(t p) q -> p t q", p=128)
            for t in range(KT):
                m8 = stagep.tile([128, S], U8, tag="m8")
                nc.sync.dma_start(m8[:, :], m_ap[:, t, :])
                nc.gpsimd.tensor_copy(m16[:, t, :], m8[:, :])

            for h in range(HPC):
                qtf = stagep.tile([D, S], F32, tag="qs")
                nc.sync.dma_start(qtf[:, :], qt_d[h])
                q16 = headp.tile([D, S], F16, tag="q16")
                nc.gpsimd.tensor_copy(q16[:, :], qtf[:, :])

                ktf = stagep.tile([D, S], F32, tag="ks")
                nc.sync.dma_start(ktf[:, :], kt_d[h])
                k16 = headp.tile([D, S], F16, tag="k16")
                nc.gpsimd.tensor_copy(k16[:, :], ktf[:, :])

                vf = stagep.tile([128, KT, D], F32, tag="vs")
                nc.sync.dma_start(vf[:, :, :], v_d[h].rearrange("(t p) d -> p t d", p=128))
                v16 = headp.tile([128, KT, D + 1], F16, tag="v16")
                nc.gpsimd.tensor_copy(v16[:, :, 0:D], vf[:, :, :])
                nc.gpsimd.memset(v16[:, :, D : D + 1], 1.0)

                for qc in range(QC):
                    ot = otp.tile([D + 1, QCS], F32)
                    for t in range(KT):
                        st = stp.tile([128, QCS], F32)
                        for half in range(QCS // 512):
                            nc.tensor.matmul(
                                st[:, ts(half, 512)],
                                k16[:, ts(t, 128)],
                                q16[:, qc * QCS + half * 512 : qc * QCS + (half + 1) * 512],
                                start=True,
                                stop=True,
                            )
                        pt = ptp.tile([128, QCS], F16)
                        nc.scalar.activation(
                            pt[:, :],
                            st[:, :],
                            mybir.ActivationFunctionType.Exp,
                            scale=float(SCALE),
                        )
                        nc.vector.tensor_mul(
                            pt[:, :], pt[:, :], m16[:, t, ts(qc, QCS)]
                        )
                        for half in range(QCS // 512):
                            nc.tensor.matmul(
                                ot[:, ts(half, 512)],
                                v16[:, t, :],
                                pt[:, ts(half, 512)],
                                start=(t == 0),
                                stop=(t == KT - 1),
                            )
                    # epilogue: normalize + transpose back to [q, d]
                    ots = epp.tile([D + 1, QCS], F32, tag="ots")
                    nc.vector.tensor_copy(ots[:, :], ot[:, :])
                    for j in range(JT):
                        tp = tpp.tile([128, D + 1], F32)
                        nc.tensor.transpose(
                            tp[:, :], ots[:, ts(j, 128)], ident[0 : D + 1, 0 : D + 1]
                        )
                        r = epp.tile([128, 1], F32, tag="r")
                        nc.vector.reciprocal(r[:, :], tp[:, D : D + 1])
                        ob = epp.tile([128, D], F32, tag="ob")
                        nc.vector.tensor_scalar_mul(ob[:, :], tp[:, 0:D], r[:, :])
                        nc.sync.dma_start(
                            o_d[h, qc * QCS + j * 128 : qc * QCS + (j + 1) * 128, :],
                            ob[:, :],
                        )
    nc.finalize()
    return nc


def make_in_maps(Q, K, V, mask):
    """Shard host-side: 8 consecutive (b,h) pairs per core; Q/K transposed
    per head so the head dim d is the SBUF partition dim; mask transposed
    to [k, q] uint8 keep-flags (shared by all cores)."""
    Qf = np.ascontiguousarray(
        Q.reshape(B * H, S, D).transpose(0, 2, 1), dtype=np.float32
    )
    Kf = np.ascontiguousarray(
        K.reshape(B * H, S, D).transpose(0, 2, 1), dtype=np.float32
    )
    Vf = np.ascontiguousarray(V.reshape(B * H, S, D), dtype=np.float32)
    keepT = np.ascontiguousarray((~np.asarray(mask[0, 0])).T).astype(np.uint8)
    in_maps = []
    for c in range(N_CORES):
        sl = slice(c * HPC, (c + 1) * HPC)
        in_maps.append(
            {"qt": Qf[sl], "kt": Kf[sl], "v": Vf[sl], "maskt": keepT}
        )
    return in_maps


_NC_CACHE = None


def get_nc():
    global _NC_CACHE
    if _NC_CACHE is None:
        _NC_CACHE = build_nc()
    return _NC_CACHE


def kernel(Q, K, V, mask):
    nc = get_nc()
    in_maps = make_in_maps(Q, K, V, mask)
    res = run_bass_kernel_spmd(nc, in_maps, core_ids=list(range(N_CORES)))
    out = np.stack([res.results[c]["o"] for c in range(N_CORES)])
    return out.reshape(B, H, S, D)


# revision 3
# speedup vs baseline: 1.4388x; 1.4388x over previous
"""Masked multi-head attention for Trainium2, SPMD across 8 NeuronCores.

Problem: Q,K,V [4,16,2048,64] f32, mask [1,1,2048,2048] bool (True = masked).
  out = softmax(QK^T/sqrt(64) masked) @ V

Sharding: B*H = 64 (batch, head) pairs -> 8 per core (data + head parallel,
no cross-core communication). Each core runs the identical program on its
8 heads.

Per-core algorithm (S=2048, D=64), matmul compute in bf16 with f32 PSUM
accumulation:
  - Host passes Q,K transposed per head ([64, 2048], d-major) so the
    contraction dim (d) lands on SBUF partitions; V natural; the mask
    transposed as bf16 keep-flags (1.0 = keep, 0.0 = masked).
  - For each head, for each q-chunk of 1024, for each k-tile of 128:
      S^T[k,q] = (K^T block [64,128]).T @ Q^T [64,1024]      (TensorE)
      P^T = exp(0.125 * S^T)  f32 PSUM -> bf16 SBUF          (ScalarE LUT)
      PM^T = P^T * maskT[k,q]                                (VectorE 2x)
      O^T[65,1024] += (V'[128,65]).T @ PM^T  (V' = [V | 1])  (TensorE)
    The ones column of V' makes O^T row 64 the softmax denominator, so no
    separate reduction pass is needed; softmax max-subtraction is skipped
    (scores ~ N(0,1): exp cannot overflow) which matches the reference to
    rounding.
  - Epilogue per q-chunk: copy O^T to SBUF, PE-transpose 128-col blocks,
    out[q, d] = O^T.T[q, 0:64] * (1 / O^T.T[q, 64]), DMA to DRAM.
"""

import ml_dtypes
import numpy as np

import concourse.bass as bass
import concourse.mybir as mybir
import concourse.tile as tile
from concourse import bacc
from concourse.bass import ts
from concourse.bass_utils import run_bass_kernel_spmd
from concourse.masks import make_identity

F32 = mybir.dt.float32
BF16 = mybir.dt.bfloat16

N_CORES = 8
B, H, S, D = 4, 16, 2048, 64
HPC = B * H // N_CORES  # heads per core = 8
KT = S // 128  # 16 k-tiles
QC = 2  # q-chunks of 1024
QCS = S // QC  # 1024
JT = QCS // 128  # 8 epilogue blocks per q-chunk
SCALE = 1.0 / np.sqrt(D)


def build_nc():
    nc = bacc.Bacc("TRN2", target_bir_lowering=False)

    qt_d = nc.declare_dram_parameter("qt", [HPC, D, S], F32, isOutput=False)
    kt_d = nc.declare_dram_parameter("kt", [HPC, D, S], F32, isOutput=False)
    v_d = nc.declare_dram_parameter("v", [HPC, S, D], F32, isOutput=False)
    m_d = nc.declare_dram_parameter("maskt", [S, S], BF16, isOutput=False)
    o_d = nc.declare_dram_parameter("o", [HPC, S, D], F32, isOutput=True)

    with tile.TileContext(nc) as tc:
        with (
            tc.tile_pool(name="const", bufs=1) as constp,
            tc.tile_pool(name="mask", bufs=1) as maskp,
            tc.tile_pool(name="stage", bufs=2) as stagep,
            tc.tile_pool(name="head", bufs=2) as headp,
            tc.tile_pool(name="pt", bufs=3) as ptp,
            tc.tile_pool(name="pm", bufs=3) as pmp,
            tc.tile_pool(name="ep", bufs=3) as epp,
            tc.tile_pool(name="st", bufs=2, space="PSUM") as stp,
            tc.tile_pool(name="ot", bufs=1, space="PSUM") as otp,
            tc.tile_pool(name="tp", bufs=2, space="PSUM") as tpp,
        ):
            ident = constp.tile([128, 128], F32)
            make_identity(nc, ident[:, :])

            # mask: bf16 keep-flags, transposed on host; DMA straight in
            m16 = maskp.tile([128, KT, S], BF16)
            m_ap = m_d.ap().rearrange("(t p) q -> p t q", p=128)
            for t in range(KT):
                nc.sync.dma_start(m16[:, t, :], m_ap[:, t, :])

            for h in range(HPC):
                qtf = stagep.tile([D, S], F32, tag="qs")
                nc.sync.dma_start(qtf[:, :], qt_d[h])
                q16 = headp.tile([D, S], BF16, tag="q16")
                nc.vector.tensor_copy(q16[:, :], qtf[:, :])

                ktf = stagep.tile([D, S], F32, tag="ks")
                nc.sync.dma_start(ktf[:, :], kt_d[h])
                k16 = headp.tile([D, S], BF16, tag="k16")
                nc.vector.tensor_copy(k16[:, :], ktf[:, :])

                vf = stagep.tile([128, KT, D], F32, tag="vs")
                nc.sync.dma_start(vf[:, :, :], v_d[h].rearrange("(t p) d -> p t d", p=128))
                v16 = headp.tile([128, KT, D + 1], BF16, tag="v16")
                nc.vector.tensor_copy(v16[:, :, 0:D], vf[:, :, :])
                nc.gpsimd.memset(v16[:, :, D : D + 1], 1.0)

                for qc in range(QC):
                    ot = otp.tile([D + 1, QCS], F32)
                    for t in range(KT):
                        st = stp.tile([128, QCS], F32)
                        for half in range(QCS // 512):
                            nc.tensor.matmul(
                                st[:, ts(half, 512)],
                                k16[:, ts(t, 128)],
                                q16[:, qc * QCS + half * 512 : qc * QCS + (half + 1) * 512],
                                start=True,
                                stop=True,
                            )
                        pt = ptp.tile([128, QCS], BF16)
                        nc.scalar.activation(
                            pt[:, :],
                            st[:, :],
                            mybir.ActivationFunctionType.Exp,
                            scale=float(SCALE),
                        )
                        pm = pmp.tile([128, QCS], BF16)
                        nc.vector.tensor_mul(
                            pm[:, :], pt[:, :], m16[:, t, ts(qc, QCS)]
                        )
                        for half in range(QCS // 512):
                            nc.tensor.matmul(
                                ot[:, ts(half, 512)],
                                v16[:, t, :],
                                pm[:, ts(half, 512)],
                                start=(t == 0),
                                stop=(t == KT - 1),
                            )
                    # epilogue: normalize + transpose back to [q, d]
                    ots = epp.tile([D + 1, QCS], F32, tag="ots")
                    nc.vector.tensor_copy(ots[:, :], ot[:, :])
                    for j in range(JT):
                        tp = tpp.tile([128, D + 1], F32)
                        nc.tensor.transpose(
                            tp[:, :], ots[:, ts(j, 128)], ident[0 : D + 1, 0 : D + 1]
                        )
                        r = epp.tile([128, 1], F32, tag="r")
                        nc.vector.reciprocal(r[:, :], tp[:, D : D + 1])
                        ob = epp.tile([128, D], F32, tag="ob")
                        nc.vector.tensor_scalar_mul(ob[:, :], tp[:, 0:D], r[:, :])
                        nc.sync.dma_start(
                            o_d[h, qc * QCS + j * 128 : qc * QCS + (j + 1) * 128, :],
                            ob[:, :],
                        )
    nc.finalize()
    return nc


def make_in_maps(Q, K, V, mask):
    """Shard host-side: 8 consecutive (b,h) pairs per core; Q/K transposed
    per head so the head dim d is the SBUF partition dim; mask transposed
    to [k, q] bf16 keep-flags (shared by all cores)."""
    Qf = np.ascontiguousarray(
        np.asarray(Q).reshape(B * H, S, D).transpose(0, 2, 1), dtype=np.float32
    )
    Kf = np.ascontiguousarray(
        np.asarray(K).reshape(B * H, S, D).transpose(0, 2, 1), dtype=np.float32
    )
    Vf = np.ascontiguousarray(np.asarray(V).reshape(B * H, S, D), dtype=np.float32)
    keepT = np.ascontiguousarray((~np.asarray(mask[0, 0])).T).astype(
        ml_dtypes.bfloat16
    )
    in_maps = []
    for c in range(N_CORES):
        sl = slice(c * HPC, (c + 1) * HPC)
        in_maps.append({"qt": Qf[sl], "kt": Kf[sl], "v": Vf[sl], "maskt": keepT})
    return in_maps


_NC_CACHE = None


def get_nc():
    global _NC_CACHE
    if _NC_CACHE is None:
        _NC_CACHE = build_nc()
    return _NC_CACHE


def kernel(Q, K, V, mask):
    nc = get_nc()
    in_maps = make_in_maps(Q, K, V, mask)
    res = run_bass_kernel_spmd(nc, in_maps, core_ids=list(range(N_CORES)))
    out = np.stack([res.results[c]["o"] for c in range(N_CORES)])
    return out.reshape(B, H, S, D)
